# revision 2
# baseline (speedup 1.0000x reference)
"""AttentionalSplatting Trainium2 kernel (8 NeuronCores, SPMD).

Sharding: 8 cores = T(4) x HW-halves(2).  Core c handles t = c//2 and pixel
columns [ (c%2)*1152, (c%2+1)*1152 ).  Each core runs the full pipeline for
its (t, pixel-half): coord-proj + 2D RoPE -> Q/K/V proj -> qk-norm ->
scores(+spatial bias) -> softmax -> attend -> Wo -> W_out -> residual.
No cross-core communication is needed; outputs concatenate.

On-chip layout is feature-major ("transposed"): feature/head dims live on
SBUF partitions, pixels/tokens on the free dim.  Scores are computed as
S^T (m on partitions, q free) so the attend matmul consumes exp(S^T)
directly and softmax sums arrive free via a ones-column appended to V.
The spatial bias enters PSUM through identity matmuls so the exp can read
(scores+bias) straight out of PSUM on the scalar engine.
"""

import math
import sys

import numpy as np

sys.path.insert(0, "/opt/trn_rl_repo")

import ml_dtypes  # noqa: E402

import concourse.bass as bass  # noqa: E402
import concourse.bacc as bacc  # noqa: E402
import concourse.tile as tile  # noqa: E402
from concourse import mybir  # noqa: E402
from concourse.bass_utils import run_bass_kernel_spmd  # noqa: E402

T, M, HW, DF, H = 4, 1024, 2304, 256, 8
DKH = DF // H  # 32
QH = HW // 2  # 1152 pixels per core
SCALE = 1.0 / math.sqrt(DKH)
D_HALF = DF // 2  # 128
D_QUART = DF // 4  # 64
THETA = (100.0 ** (-2.0 * np.arange(D_QUART, dtype=np.float32) / D_HALF)).astype(
    np.float32
)

F32 = mybir.dt.float32
BF16 = mybir.dt.bfloat16
AF = mybir.ActivationFunctionType
BF16NP = ml_dtypes.bfloat16

N_CORES = 8
Q_BLOCKS = [(0, 512), (512, 512), (1024, 128)]
M_TRIPS = [(0, 1, 2), (3, 4, 5), (6, 7)]
K_CHUNKS = [(0, 512), (512, 512)]


def _bf(x):
    return np.ascontiguousarray(np.asarray(x, np.float32)).astype(BF16NP)


def _f32(x):
    return np.ascontiguousarray(np.asarray(x, np.float32))


def _host_constants(Wq, Wk, Wv, Wo, W_out_w, W_out_b, W_coord_w, W_coord_b):
    """Shared (core-independent) device constants, host-precomputed."""
    # pair-swapped coord weights for RoPE (swap even/odd output columns)
    perm = np.arange(DF)
    perm = perm.reshape(-1, 2)[:, ::-1].reshape(-1)
    wcsw = W_coord_w[:, perm]
    wcbsw = W_coord_b[perm]
    # signed duplicated theta: [-t0, +t0, -t1, +t1, ...]
    th = np.empty((1, D_HALF), np.float32)
    th[0, 0::2] = -THETA
    th[0, 1::2] = THETA
    # block-ones for per-head sum of squares: dtile k maps its 128 feature
    # rows onto head columns 4k..4k+3
    bones = np.zeros((2, 128, 8), np.float32)
    for k in range(2):
        for d in range(128):
            bones[k, d, 4 * k + d // 32] = 1.0
    # expand per-head scalars (8, q) back to the 128 feature rows of dtile k
    exp8 = np.zeros((2, 8, 128), np.float32)
    for k in range(2):
        for d in range(128):
            exp8[k, 4 * k + d // 32, d] = 1.0
    # expand per-head inv-sum (8, q) to paired attend-output rows:
    # pair j holds head 2j at rows 1..33 and head 2j+1 at rows 65..97
    expP = np.zeros((4, 8, 128), np.float32)
    for j in range(4):
        expP[j, 2 * j, 1:33] = 1.0
        expP[j, 2 * j + 1, 65:97] = 1.0
    # Wo rearranged to the paired attend-output row layout (sumexp rows = 0)
    wo_aug = np.zeros((4, 128, DF), np.float32)
    for j in range(4):
        wo_aug[j, 1:33, :] = Wo[(2 * j) * 32 : (2 * j + 1) * 32, :]
        wo_aug[j, 65:97, :] = Wo[(2 * j + 1) * 32 : (2 * j + 2) * 32, :]
    return {
        "wq": _bf(Wq),
        "wk": _bf(Wk),
        "wv": _bf(Wv),
        "wo_aug": _bf(wo_aug),
        "wout": _bf(W_out_w),
        "woutb": _f32(W_out_b).reshape(DF, 1),
        "wc": _f32(W_coord_w),
        "wcb": _f32(W_coord_b).reshape(DF, 1),
        "wcsw": _f32(wcsw),
        "wcbsw": _f32(wcbsw).reshape(DF, 1),
        "theta2s": th,
        "bones": bones,
        "exp8": exp8,
        "expP": expP,
        "ident": _bf(np.eye(128, dtype=np.float32)),
    }


_NC_CACHE = None


def _build_nc():
    nc = bacc.Bacc(
        "TRN2",
        target_bir_lowering=False,
        debug=False,
        enable_asserts=True,
        num_devices=N_CORES,
    )
    d = {}
    inp = lambda name, shape, dt: d.__setitem__(
        name, nc.declare_dram_parameter(name, list(shape), dt, isOutput=False)
    )
    inp("tokT", (DF, M), BF16)
    inp("posT", (2, QH), F32)
    inp("biasT", (M, QH), BF16)
    inp("fmapT", (DF, QH), F32)
    inp("wq", (DF, DF), BF16)
    inp("wk", (DF, DF), BF16)
    inp("wv", (DF, DF), BF16)
    inp("wo_aug", (4, 128, DF), BF16)
    inp("wout", (DF, DF), BF16)
    inp("woutb", (DF, 1), F32)
    inp("wc", (2, DF), F32)
    inp("wcb", (DF, 1), F32)
    inp("wcsw", (2, DF), F32)
    inp("wcbsw", (DF, 1), F32)
    inp("theta2s", (1, D_HALF), F32)
    inp("bones", (2, 128, 8), F32)
    inp("exp8", (2, 8, 128), F32)
    inp("expP", (4, 8, 128), F32)
    inp("ident", (128, 128), BF16)
    out = nc.declare_dram_parameter("out", [DF, QH], F32, isOutput=True)

    import os as _os

    with tile.TileContext(
        nc, trace_sim=bool(_os.environ.get("KERNEL_TRACE_SIM"))
    ) as tc:
        _body(nc, tc, d, out)
    nc.compile()
    return nc


def _body(nc, tc, d, out_dram):
    mm = nc.tensor.matmul
    act = nc.scalar.activation
    dma = nc.sync.dma_start

    with (
        tc.tile_pool(name="const", bufs=1) as cpool,
        tc.tile_pool(name="work", bufs=1) as wpool,
        tc.tile_pool(name="persist", bufs=1) as ppool,
        tc.tile_pool(name="epool", bufs=4) as epool,
        tc.tile_pool(name="psA", bufs=2, space=bass.MemorySpace.PSUM) as psA,
        tc.tile_pool(name="psB", bufs=2, space=bass.MemorySpace.PSUM) as psB,
    ):
        # ---- load constants / inputs to SBUF ----
        # 256-row tensors fold to (128, 2, ...): [:, kt, ...] = rows kt*128..
        def load(name, shape, dt, rearrange=None, **kw):
            t = cpool.tile(list(shape), dt, tag=name)
            src = d[name][:]
            if rearrange is not None:
                src = src.rearrange(rearrange, **kw)
            dma(t[:], src)
            return t

        fold = "(k p) d -> p k d"
        wq = load("wq", (128, 2, DF), BF16, fold, p=128)
        wk = load("wk", (128, 2, DF), BF16, fold, p=128)
        wv = load("wv", (128, 2, DF), BF16, fold, p=128)
        wo_aug = load("wo_aug", (128, 4, DF), BF16, "j p d -> p j d")
        wout = load("wout", (128, 2, DF), BF16, fold, p=128)
        woutb = load("woutb", (128, 2, 1), F32, fold, p=128)
        wc = load("wc", (2, DF), F32)
        wcb = load("wcb", (128, 2, 1), F32, fold, p=128)
        wcsw = load("wcsw", (2, DF), F32)
        wcbsw = load("wcbsw", (128, 2, 1), F32, fold, p=128)
        th2 = load("theta2s", (1, D_HALF), F32)
        bones = load("bones", (128, 2, 8), F32, "k p h -> p k h")
        exp8 = load("exp8", (8, 2, 128), F32, "k h d -> h k d")
        expP = load("expP", (8, 4, 128), F32, "j s e -> s j e")
        ident = load("ident", (128, 128), BF16)
        posT = load("posT", (2, QH), F32)
        posT2 = cpool.tile([1, 2, QH], F32, tag="posT2")
        dma(posT2[:], d["posT"][:].rearrange("(o a) q -> o a q", o=1))
        fmapT = load("fmapT", (128, 2, QH), F32, fold, p=128)
        tokT = load("tokT", (128, 2, M), BF16, fold, p=128)

        bias_sb = []
        for mc in range(8):
            bt = ppool.tile([128, QH], BF16, tag=f"bias{mc}")
            dma(bt[:], d["biasT"][mc * 128 : (mc + 1) * 128, :])
            bias_sb.append(bt)

        def const_tile(val, name):
            t = cpool.tile([128, 1], F32, tag=name)
            nc.vector.memset(t[:], val)
            return t

        halfpi = const_tile(math.pi / 2.0, "halfpi")
        zero_c = const_tile(0.0, "zeroc")
        lnscale = const_tile(math.log(SCALE), "lnscale")

        # ---- Q path: Qin^T (and pair-swapped) = Wc^T @ pos^T (+b) ----
        qin = []
        for dt_i, (w, b) in enumerate(((wc, wcb), (wcsw, wcbsw))):
            for half in range(2):
                ps = psA.tile([128, QH], F32, tag="big")
                for qo, qb in Q_BLOCKS:
                    mm(
                        ps[:, qo : qo + qb],
                        w[:, half * 128 : (half + 1) * 128],
                        posT[:, qo : qo + qb],
                    )
                t = wpool.tile([128, QH], BF16, tag=f"qin{dt_i}{half}")
                nc.vector.tensor_scalar_add(t[:], ps[:], b[:, half, :])
                qin.append(t)
        qin0, qin1, qins0, qins1 = qin

        # ---- RoPE tables: one angle matmul per axis (signed theta) ----
        cs = []
        for axis in range(2):
            ps = psA.tile([128, QH], F32, tag="big")
            for qo, qb in Q_BLOCKS:
                mm(
                    ps[:, qo : qo + qb],
                    th2[:, :],
                    posT2[:, axis, qo : qo + qb],
                )
            c_t = wpool.tile([128, QH], BF16, tag=f"cos{axis}")
            s_t = wpool.tile([128, QH], BF16, tag=f"sin{axis}")
            act(c_t[:], ps[:], AF.Sin, bias=halfpi[:])
            act(s_t[:], ps[:], AF.Sin)
            cs.append((c_t, s_t))

        roped = []
        for dt_i, (q, qs) in enumerate(((qin0, qins0), (qin1, qins1))):
            c_t, s_t = cs[dt_i]
            t1 = wpool.tile([128, QH], BF16, tag=f"ropea{dt_i}")
            nc.vector.tensor_mul(t1[:], q[:], c_t[:])
            t2 = wpool.tile([128, QH], BF16, tag=f"ropeb{dt_i}")
            nc.vector.tensor_mul(t2[:], qs[:], s_t[:])
            r = wpool.tile([128, QH], BF16, tag=f"roped{dt_i}")
            nc.vector.tensor_add(r[:], t1[:], t2[:])
            roped.append(r)

        # ---- Q = roped @ Wq  (computed as Q^T, feature-major) ----
        def proj_T(w_sb, rhs_tiles, n, blocks, name):
            """out^T[dt] (128, n) = sum_kt w[kt,dt]^T @ rhs[kt]; returns psum tiles"""
            outs = []
            for dt_i in range(2):
                ps = psA.tile([128, n], F32, tag="big")
                for qo, qb in blocks:
                    for kt in range(2):
                        mm(
                            ps[:, qo : qo + qb],
                            w_sb[:, kt, dt_i * 128 : (dt_i + 1) * 128],
                            rhs_tiles[kt][:, qo : qo + qb],
                            start=(kt == 0),
                            stop=(kt == 1),
                        )
                outs.append(ps)
            return outs

        def qknorm(ps_list, n, blocks, ln_bias, name):
            """psum (128, n) x2 -> normalized bf16 tiles (128, n) x2"""
            sq_ps = psA.tile([8, n], F32, tag="big")
            bf_tiles = []
            for dt_i, ps in enumerate(ps_list):
                tb = ppool.tile([128, n], BF16, tag=f"{name}n{dt_i}")
                nc.vector.tensor_copy(tb[:], ps[:])
                sq = wpool.tile([128, n], F32, tag=f"{name}f{dt_i}")
                nc.vector.tensor_copy(sq[:], ps[:])
                nc.vector.tensor_mul(sq[:], sq[:], sq[:])
                for qo, qb in blocks:
                    mm(
                        sq_ps[:, qo : qo + qb],
                        bones[:, dt_i, :],
                        sq[:, qo : qo + qb],
                        start=(dt_i == 0),
                        stop=(dt_i == 1),
                    )
                bf_tiles.append(tb)
            lnt = wpool.tile([8, n], F32, tag=f"{name}ln")
            act(lnt[:], sq_ps[:], AF.Ln)
            if ln_bias is None:
                ln_bias = zero_c
            invn = wpool.tile([8, n], F32, tag=f"{name}inv")
            act(invn[:], lnt[:], AF.Exp, scale=-0.5, bias=ln_bias[:8, :])
            outs = []
            for dt_i, tb in enumerate(bf_tiles):
                psx = psA.tile([128, n], F32, tag="big")
                for qo, qb in blocks:
                    mm(psx[:, qo : qo + qb], exp8[:, dt_i, :], invn[:, qo : qo + qb])
                tn = ppool.tile([128, n], BF16, tag=f"{name}T{dt_i}")
                nc.vector.tensor_mul(tn[:], tb[:], psx[:])
                outs.append(tn)
            return outs

        q_ps = proj_T(wq, roped, QH, Q_BLOCKS, "q")
        qnT = qknorm(q_ps, QH, Q_BLOCKS, lnscale, "q")

        tok_tiles = [tokT[:, 0, :], tokT[:, 1, :]]
        k_ps = proj_T(wk, tok_tiles, M, K_CHUNKS, "k")
        knT = qknorm(k_ps, M, K_CHUNKS, None, "k")

        # ---- V (token-major) with ones column:  vsb[mc] = (128, 8, 33) ----
        vsb = []
        for mc in range(8):
            ps = psB.tile([128, 256], F32, tag="small")
            for kt in range(2):
                mm(
                    ps[:],
                    tokT[:, kt, mc * 128 : (mc + 1) * 128],
                    wv[:, kt, :],
                    start=(kt == 0),
                    stop=(kt == 1),
                )
            vt = ppool.tile([128, 8, 33], BF16, tag=f"v{mc}")
            nc.vector.memset(vt[:, :, 0:1], 1.0)
            nc.vector.tensor_copy(
                vt[:, :, 1:33], ps[:].rearrange("p (h e) -> p h e", h=8)
            )
            vsb.append(vt)

        # ---- main attention loop ----
        # pair j: head 2j accumulates at psum rows 0..32, head 2j+1 at 64..96
        osb = []  # per pair (128, QH) bf16, rows 0/64 = sumexp
        for j in range(4):
            t = ppool.tile([128, QH], BF16, tag=f"osb{j}")
            osb.append(t)

        for qo, qb in Q_BLOCKS:
            for j in range(4):
                heads = (2 * j, 2 * j + 1)
                o_ps = psB.tile([128, qb], F32, tag="small")
                for trip in M_TRIPS:
                    w3 = len(trip) * qb
                    e_ts = {}
                    s_tiles = {}
                    for h in heads:
                        dt_i = h // 4
                        hp = (h % 4) * 32
                        s_ps = psA.tile([128, w3], F32, tag="big")
                        s_tiles[h] = s_ps
                        for i, mc in enumerate(trip):
                            mm(
                                s_ps[:, i * qb : (i + 1) * qb],
                                ident[:],
                                bias_sb[mc][:, qo : qo + qb],
                                start=True,
                                stop=False,
                            )
                    for i, mc in enumerate(trip):
                        for h in heads:
                            dt_i = h // 4
                            hp = (h % 4) * 32
                            mm(
                                s_tiles[h][:, i * qb : (i + 1) * qb],
                                knT[dt_i][hp : hp + 32, mc * 128 : (mc + 1) * 128],
                                qnT[dt_i][hp : hp + 32, qo : qo + qb],
                                start=False,
                                stop=True,
                                tile_position=(hp, 0),
                            )
                    for h in heads:
                        e_t = epool.tile([128, 3 * qb], BF16, tag="E")
                        act(e_t[:, 0:w3], s_tiles[h][:], AF.Exp)
                        e_ts[h] = e_t
                    for i, mc in enumerate(trip):
                        for h in heads:
                            base = 64 * (h % 2)
                            mm(
                                o_ps[base : base + 33, :],
                                vsb[mc][:, h, :],
                                e_ts[h][:, i * qb : (i + 1) * qb],
                                start=(mc == 0),
                                stop=(mc == 7),
                                tile_position=(0, base),
                            )
                nc.vector.tensor_copy(osb[j][:, qo : qo + qb], o_ps[:])

        # ---- softmax denominators: gather row 0 of each head, invert ----
        sumE = wpool.tile([8, QH], BF16, tag="sumE")
        for h in range(8):
            j, r = h // 2, 64 * (h % 2)
            dma(sumE[h : h + 1, :], osb[h // 2][r : r + 1, :])
        lnS = wpool.tile([8, QH], F32, tag="lnS")
        act(lnS[:], sumE[:], AF.Ln)
        invS = wpool.tile([8, QH], F32, tag="invS")
        act(invS[:], lnS[:], AF.Exp, scale=-1.0)

        for j in range(4):
            for qo, qb in Q_BLOCKS:
                ps = psB.tile([128, qb], F32, tag="small")
                mm(ps[:], expP[:, j, :], invS[:, qo : qo + qb])
                nc.vector.tensor_mul(
                    osb[j][:, qo : qo + qb], osb[j][:, qo : qo + qb], ps[:]
                )

        # ---- output projections + residual ----
        o1b = []
        for dt_i in range(2):
            ps = psA.tile([128, QH], F32, tag="big")
            for qo, qb in Q_BLOCKS:
                for j in range(4):
                    mm(
                        ps[:, qo : qo + qb],
                        wo_aug[:, j, dt_i * 128 : (dt_i + 1) * 128],
                        osb[j][:, qo : qo + qb],
                        start=(j == 0),
                        stop=(j == 3),
                    )
            t = wpool.tile([128, QH], BF16, tag=f"o1b{dt_i}")
            nc.vector.tensor_copy(t[:], ps[:])
            o1b.append(t)

        for dt_i in range(2):
            ps = psA.tile([128, QH], F32, tag="big")
            for qo, qb in Q_BLOCKS:
                for kt in range(2):
                    mm(
                        ps[:, qo : qo + qb],
                        wout[:, kt, dt_i * 128 : (dt_i + 1) * 128],
                        o1b[kt][:, qo : qo + qb],
                        start=(kt == 0),
                        stop=(kt == 1),
                    )
            r1 = wpool.tile([128, QH], F32, tag=f"res{dt_i}")
            nc.vector.tensor_scalar_add(r1[:], ps[:], woutb[:, dt_i, :])
            nc.vector.tensor_add(r1[:], r1[:], fmapT[:, dt_i, :])
            dma(out_dram[dt_i * 128 : (dt_i + 1) * 128, :], r1[:])


def kernel(
    track_tokens,
    feature_map,
    feature_positions,
    spatial_bias,
    Wq,
    Wk,
    Wv,
    Wo,
    W_out_w,
    W_out_b,
    W_coord_w,
    W_coord_b,
):
    global _NC_CACHE
    inputs = dict(
        track_tokens=track_tokens,
        feature_map=feature_map,
        feature_positions=feature_positions,
        spatial_bias=spatial_bias,
        Wq=Wq,
        Wk=Wk,
        Wv=Wv,
        Wo=Wo,
        W_out_w=W_out_w,
        W_out_b=W_out_b,
        W_coord_w=W_coord_w,
        W_coord_b=W_coord_b,
    )
    in_maps = build_in_maps(inputs)
    if _NC_CACHE is None:
        _NC_CACHE = _build_nc()
    res = run_bass_kernel_spmd(_NC_CACHE, in_maps, core_ids=list(range(N_CORES)))
    return assemble_output([res.results[c]["out"] for c in range(N_CORES)])


def build_in_maps(inputs):
    consts = _host_constants(
        np.asarray(inputs["Wq"], np.float32),
        np.asarray(inputs["Wk"], np.float32),
        np.asarray(inputs["Wv"], np.float32),
        np.asarray(inputs["Wo"], np.float32),
        np.asarray(inputs["W_out_w"], np.float32),
        np.asarray(inputs["W_out_b"], np.float32),
        np.asarray(inputs["W_coord_w"], np.float32),
        np.asarray(inputs["W_coord_b"], np.float32),
    )
    track_tokens = np.asarray(inputs["track_tokens"], np.float32)
    feature_map = np.asarray(inputs["feature_map"], np.float32)
    feature_positions = np.asarray(inputs["feature_positions"], np.float32)
    spatial_bias = np.asarray(inputs["spatial_bias"], np.float32)

    in_maps = []
    for c in range(N_CORES):
        t, half = c // 2, c % 2
        qsl = slice(half * QH, (half + 1) * QH)
        m = dict(consts)
        m["tokT"] = _bf(track_tokens[t].T)
        m["posT"] = _f32(feature_positions[t, qsl].T)
        m["biasT"] = _bf(spatial_bias[t][:, qsl])
        m["fmapT"] = _f32(feature_map[t, qsl].T)
        in_maps.append(m)
    return in_maps


def assemble_output(per_core):
    """per_core: sequence of 8 per-core 'out' arrays, each (DF, QH)."""
    out = np.empty((T, HW, DF), np.float32)
    for c in range(N_CORES):
        t, half = c // 2, c % 2
        qsl = slice(half * QH, (half + 1) * QH)
        out[t, qsl, :] = np.asarray(per_core[c]).T
    return out



# revision 6
# speedup vs baseline: 48.3724x; 48.3724x over previous
"""AttentionalSplatting Trainium2 kernel v2 (8 NeuronCores, SPMD).

Sharding: 8 cores = T(4) x HW-halves(2).  Core c handles t = c//2 and pixel
columns [(c%2)*1152, (c%2+1)*1152).  No cross-core communication.

v2 redesign vs v1:
- spatial bias enters as host-precomputed exp(bias); attention weights are
  exp(s)*expB via DVE/Pool multiplies instead of PE identity-matmul
  injection (saves ~74k PE cycles/core).
- scores matmuls run in fp8e4m3 DoubleRow (2 k-tiles of 16 partitions),
  halving score streaming time. sqrt(SCALE) is folded into both Q and K
  during qk-norm so fp8 operands stay in normal range.
- Wo and W_out_w are fused into one matrix on the host.
- coordinate projection uses an exact hi/lo split of positions in bf16
  (integer part exact, fractional in [-0.5,0.5]) with the coord bias as a
  fifth contraction row; only the RoPE angle outer-product stays f32.
- main loop blocks pixels by 384 so each Act exp instruction covers
  (2 heads x 384) rows of one PSUM tile, reducing per-instruction Act
  access bubbles.
"""

import math
import sys

import numpy as np

sys.path.insert(0, "/opt/trn_rl_repo")

import ml_dtypes  # noqa: E402

import concourse.bass as bass  # noqa: E402
import concourse.bacc as bacc  # noqa: E402
import concourse.tile as tile  # noqa: E402
from concourse import mybir  # noqa: E402
from concourse.bass_utils import run_bass_kernel_spmd  # noqa: E402

T, M, HW, DF, H = 4, 1024, 2304, 256, 8
DKH = DF // H  # 32
QH = HW // 2  # 1152 pixels per core
SCALE = 1.0 / math.sqrt(DKH)
RSC = math.sqrt(SCALE)
D_HALF = DF // 2  # 128
D_QUART = DF // 4  # 64
THETA = (100.0 ** (-2.0 * np.arange(D_QUART, dtype=np.float32) / D_HALF)).astype(
    np.float32
)

F32 = mybir.dt.float32
BF16 = mybir.dt.bfloat16
FP8 = mybir.dt.float8e4
AF = mybir.ActivationFunctionType
DR = mybir.MatmulPerfMode.DoubleRow
BF16NP = ml_dtypes.bfloat16

N_CORES = 8
QB = 384
Q_BLOCKS = [(0, 384), (384, 384), (768, 384)]
K_BLOCKS = [(0, 384), (384, 384), (768, 256)]

USE_FP8 = True
USE_BRD = True
USE_GPD = True


def _bf(x):
    return np.ascontiguousarray(np.asarray(x, np.float32)).astype(BF16NP)


def _f32(x):
    return np.ascontiguousarray(np.asarray(x, np.float32))


def _host_constants(Wq, Wk, Wv, Wo, W_out_w, W_out_b, W_coord_w, W_coord_b):
    """Shared (core-independent) device constants, host-precomputed."""
    # pair-swapped coord weights for RoPE (swap even/odd output columns)
    perm = np.arange(DF)
    perm = perm.reshape(-1, 2)[:, ::-1].reshape(-1)
    wcsw = W_coord_w[:, perm]
    wcbsw = W_coord_b[perm]
    # coord projection lhsT with hi/lo rows + bias row:
    # rhs rows are [hi_x, lo_x, hi_y, lo_y, ones]
    wc5 = np.stack(
        [W_coord_w[0], W_coord_w[0], W_coord_w[1], W_coord_w[1], W_coord_b]
    )
    wc5s = np.stack([wcsw[0], wcsw[0], wcsw[1], wcsw[1], wcbsw])
    # signed duplicated theta: [-t0, +t0, -t1, +t1, ...], split into an
    # exact bf16 hi part + bf16 lo remainder so the angle outer product can
    # run as a bf16 matmul without precision loss; plus a pi/2 row (cos).
    ths = np.empty((D_HALF,), np.float32)
    ths[0::2] = -THETA
    ths[1::2] = THETA
    th_h = ths.astype(BF16NP).astype(np.float32)
    th_l = ths - th_h
    th = np.stack([th_h, th_h, th_l, th_l, np.full((D_HALF,), np.pi / 2.0)])
    # block-ones for per-head sum of squares: dtile k maps its 128 feature
    # rows onto head columns 4k..4k+3
    bones = np.zeros((2, 128, 8), np.float32)
    for k in range(2):
        for d in range(128):
            bones[k, d, 4 * k + d // 32] = 1.0
    # expand per-head scalars (8, q) back to the 128 feature rows of dtile k,
    # scaled by sqrt(SCALE) so fp8 Q and K each carry half the 1/sqrt(dk)
    exp8 = np.zeros((2, 8, 128), np.float32)
    for k in range(2):
        for d in range(128):
            exp8[k, 4 * k + d // 32, d] = RSC
    # selector: extract attend-psum rows 0 and 64 (the ones-column sums)
    sel = np.zeros((128, 2), np.float32)
    sel[0, 0] = 1.0
    sel[64, 1] = 1.0
    # expand the two per-pair inv-sums to paired attend-output rows
    expP2 = np.zeros((2, 128), np.float32)
    expP2[0, 1:33] = 1.0
    expP2[1, 65:97] = 1.0
    # fused output projection W2 = Wo @ W_out_w in the paired attend-output
    # row layout (sumexp rows contribute 0)
    W2 = Wo @ W_out_w
    w2aug = np.zeros((4, 128, DF), np.float32)
    for j in range(4):
        w2aug[j, 1:33, :] = W2[(2 * j) * 32 : (2 * j + 1) * 32, :]
        w2aug[j, 65:97, :] = W2[(2 * j + 1) * 32 : (2 * j + 2) * 32, :]
    return {
        "wq": _bf(Wq),
        "wk": _bf(Wk),
        "wv": _bf(Wv),
        "w2aug": _bf(w2aug),
        "woutb": _f32(W_out_b).reshape(DF, 1),
        "wc5": _bf(wc5),
        "wc5s": _bf(wc5s),
        "thc": _bf(th),
        "bones": _bf(bones),
        "exp8": _bf(exp8),
        "sel": _bf(sel),
        "expP2": _bf(expP2),
    }


_NC_CACHE = None


def _build_nc(reps=1):
    nc = bacc.Bacc(
        "TRN2",
        target_bir_lowering=False,
        debug=False,
        enable_asserts=True,
        num_devices=N_CORES,
    )
    d = {}
    inp = lambda name, shape, dt: d.__setitem__(
        name, nc.declare_dram_parameter(name, list(shape), dt, isOutput=False)
    )
    inp("tokT", (DF, M), BF16)
    inp("posA", (2, 5, QH), BF16)
    inp("posHL", (5, QH), BF16)
    inp("expB", (M, QH), BF16)
    inp("fmapT", (DF, QH), F32)
    inp("wq", (DF, DF), BF16)
    inp("wk", (DF, DF), BF16)
    inp("wv", (DF, DF), BF16)
    inp("w2aug", (4, 128, DF), BF16)
    inp("woutb", (DF, 1), F32)
    inp("wc5", (5, DF), BF16)
    inp("wc5s", (5, DF), BF16)
    inp("thc", (5, D_HALF), BF16)
    inp("bones", (2, 128, 8), BF16)
    inp("exp8", (2, 8, 128), BF16)
    inp("sel", (128, 2), BF16)
    inp("expP2", (2, 128), BF16)
    out = nc.declare_dram_parameter("out", [DF, QH], F32, isOutput=True)

    import os as _os

    with tile.TileContext(
        nc, trace_sim=bool(_os.environ.get("KERNEL_TRACE_SIM"))
    ) as tc:
        for r in range(reps):
            _body(nc, tc, d, out, pfx=f"r{r}_" if reps > 1 else "")
    nc.compile()
    return nc


def _brd2(ap):
    """Broadcast an AP's (p, n) view to (p, 2, n) with a stride-0 middle dim."""
    return bass.AP(ap.tensor, ap.offset, [ap.ap[0], [0, 2], ap.ap[-1]])


def _body(nc, tc, d, out_dram, pfx=""):
    mm = nc.tensor.matmul
    act = nc.scalar.activation
    dma = nc.sync.dma_start
    dmag = nc.gpsimd.dma_start

    sdt = FP8 if USE_FP8 else BF16

    with (
        tc.tile_pool(name=pfx + "const", bufs=1) as cpool,
        tc.tile_pool(name=pfx + "work", bufs=1) as wpool,
        tc.tile_pool(name=pfx + "persist", bufs=1) as ppool,
        tc.tile_pool(name=pfx + "epool", bufs=3) as epool,
        tc.tile_pool(name=pfx + "apool", bufs=3) as apool,
        tc.tile_pool(name=pfx + "psA", bufs=2, space=bass.MemorySpace.PSUM) as psA,
        tc.tile_pool(name=pfx + "psB", bufs=2, space=bass.MemorySpace.PSUM) as psB,
    ):
        # ---- load constants / inputs to SBUF ----
        def load(name, shape, dt, rearrange=None, q=None, **kw):
            t = cpool.tile(list(shape), dt, tag=name)
            src = d[name][:]
            if rearrange is not None:
                src = src.rearrange(rearrange, **kw)
            (q or dma)(t[:], src)
            return t

        fold = "(k p) d -> p k d"
        th2 = load("theta2s", (1, D_HALF), F32)
        posT2 = cpool.tile([1, 2, QH], F32, tag=pfx + "posT2")
        dma(posT2[:], d["posT"][:].rearrange("(o a) q -> o a q", o=1))
        posHL = load("posHL", (5, QH), BF16)
        wc5 = load("wc5", (5, DF), BF16)
        wc5s = load("wc5s", (5, DF), BF16)
        tokT = load("tokT", (128, 2, M), BF16, fold, p=128)
        wq = load("wq", (128, 2, DF), BF16, fold, p=128)
        wk = load("wk", (128, 2, DF), BF16, fold, p=128)
        wv = load("wv", (128, 2, DF), BF16, fold, p=128)
        bones = load("bones", (128, 2, 8), BF16, "k p h -> p k h")
        exp8 = load("exp8", (8, 2, 128), BF16, "k h d -> h k d")
        sel = load("sel", (128, 2), BF16)
        expP2 = load("expP2", (2, 128), BF16)
        w2aug = load("w2aug", (128, 4, DF), BF16, "j p d -> p j d")
        woutb = load("woutb", (128, 2, 1), F32, fold, p=128)
        fmapT = load("fmapT", (128, 2, QH), F32, fold, p=128)

        expB_sb = []
        for mc in range(8):
            bt = ppool.tile([128, QH], BF16, tag=pfx + f"expB{mc}")
            dma(bt[:], d["expB"][mc * 128 : (mc + 1) * 128, :])
            expB_sb.append(bt)

        halfpi = cpool.tile([128, 1], F32, tag=pfx + "halfpi")
        nc.vector.memset(halfpi[:], math.pi / 2.0)

        # ---- V (token-major) with ones column: vsb[mc] = (128, 8, 33) ----
        vsb = []
        for mc in range(8):
            ps = psB.tile([128, 256], F32, tag="ops")
            for kt in range(2):
                mm(
                    ps[:],
                    tokT[:, kt, mc * 128 : (mc + 1) * 128],
                    wv[:, kt, :],
                    start=(kt == 0),
                    stop=(kt == 1),
                )
            vt = ppool.tile([128, 8, 33], BF16, tag=pfx + f"v{mc}")
            nc.vector.memset(vt[:, :, 0:1], 1.0)
            nc.vector.tensor_copy(
                vt[:, :, 1:33], ps[:].rearrange("p (h e) -> p h e", h=8)
            )
            vsb.append(vt)

        # ---- Q path: coord proj (hi/lo bf16) + 2D RoPE ----
        roped = []
        for dt_i in range(2):
            r = ppool.tile([128, QH], BF16, tag=pfx + f"roped{dt_i}")
            roped.append(r)
        for dt_i in range(2):
            csl = slice(dt_i * 128, (dt_i + 1) * 128)
            for qo, qb in Q_BLOCKS:
                # angle outer product for this axis (=dt half), f32
                ang = psB.tile([128, QB], F32, tag="ops")
                mm(ang[:, :qb], th2[:, :], posT2[:, dt_i, qo : qo + qb])
                cs = wpool.tile([128, 2, QB], BF16, tag=pfx + f"cs{dt_i}")
                act(cs[:, 0, :qb], ang[:, :qb], AF.Sin, bias=halfpi[:])
                act(cs[:, 1, :qb], ang[:, :qb], AF.Sin)
                # qin (normal, swapped) for this feature half
                qin = psA.tile([128, 2, 512], F32, tag="big")
                mm(qin[:, 0, :qb], wc5[:, csl], posHL[:, qo : qo + qb])
                mm(qin[:, 1, :qb], wc5s[:, csl], posHL[:, qo : qo + qb])
                t1 = wpool.tile([128, QB], BF16, tag=pfx + "ropea")
                nc.vector.tensor_mul(t1[:, :qb], qin[:, 0, :qb], cs[:, 0, :qb])
                t2 = wpool.tile([128, QB], BF16, tag=pfx + "ropeb")
                nc.vector.tensor_mul(t2[:, :qb], qin[:, 1, :qb], cs[:, 1, :qb])
                nc.gpsimd.tensor_add(
                    roped[dt_i][:, qo : qo + qb], t1[:, :qb], t2[:, :qb]
                )

        # ---- projections + qk-norm -> fp8 DoubleRow layouts ----

        def qk_pipeline(rhs_tiles, w_sb, blocks, name, maxb):
            """Project + qk-norm.  Returns per-half fp8 DoubleRow tiles
            dr8[dt] of shape (128, 2, n): kt0 slot holds the dense normalized
            values; kt1 slot is filled by partition-shift DMAs below."""
            n = sum(b for _, b in blocks)
            dr8 = [
                ppool.tile(
                    [128, 2, n], sdt, tag=pfx + f"{name}8_{g}",
                    name=pfx + f"{name}8_{g}",
                )
                for g in range(2)
            ]
            tb = ppool.tile([128, 2, n], BF16, tag=pfx + f"{name}tb")
            for qo, qb in blocks:
                ps = psA.tile([128, 2, maxb], F32, tag="big")
                for dt_i in range(2):
                    for kt in range(2):
                        mm(
                            ps[:, dt_i, :qb],
                            w_sb[:, kt, dt_i * 128 : (dt_i + 1) * 128],
                            rhs_tiles[kt][:, qo : qo + qb],
                            start=(kt == 0),
                            stop=(kt == 1),
                        )
                nc.vector.tensor_copy(tb[:, :, qo : qo + qb], ps[:, :, :qb])
                sq = wpool.tile([128, 2, maxb], BF16, tag=pfx + f"{name}sq")
                nc.gpsimd.tensor_mul(
                    sq[:, :, :qb], tb[:, :, qo : qo + qb], tb[:, :, qo : qo + qb]
                )
                ss = psB.tile([8, maxb], F32, tag="ss")
                for dt_i in range(2):
                    mm(
                        ss[:, :qb],
                        bones[:, dt_i, :],
                        sq[:, dt_i, :qb],
                        start=(dt_i == 0),
                        stop=(dt_i == 1),
                    )
                nrm = wpool.tile([8, maxb], F32, tag=pfx + f"{name}nrm")
                act(nrm[:, :qb], ss[:, :qb], AF.Sqrt)
                inv = wpool.tile([8, maxb], BF16, tag=pfx + f"{name}inv")
                with nc.allow_low_precision(reason="bf16 inv-norm feeds bf16 matmul"):
                    nc.vector.reciprocal(inv[:, :qb], nrm[:, :qb])
                psx = psA.tile([128, 2, maxb], F32, tag="big")
                for dt_i in range(2):
                    mm(psx[:, dt_i, :qb], exp8[:, dt_i, :], inv[:, :qb])
                for dt_i in range(2):
                    nc.vector.tensor_mul(
                        dr8[dt_i][:, 0, qo : qo + qb],
                        tb[:, dt_i, qo : qo + qb],
                        psx[:, dt_i, :qb],
                    )
            # partition-shift DMAs: dk 16..31 (partitions 32h+16..32h+32 of
            # the dense kt0 slot) move to the kt1 slot of partitions 32h..+16
            if USE_FP8:
                for g in range(2):
                    for hh in range(4):
                        p0 = 32 * hh
                        dmag(
                            dr8[g][p0 : p0 + 16, 1, :],
                            dr8[g][p0 + 16 : p0 + 32, 0, :],
                        )
            return dr8

        qn8 = qk_pipeline(roped, wq, Q_BLOCKS, "qn", QB)
        tok_tiles = [tokT[:, 0, :], tokT[:, 1, :]]
        kn8 = qk_pipeline(tok_tiles, wk, K_BLOCKS, "kn", QB)

        # ---- main attention loop ----
        osb = [
            ppool.tile([128, QH], BF16, tag=pfx + f"osb{j}", name=pfx + f"osb{j}")
            for j in range(4)
        ]
        sums = wpool.tile([8, QH], BF16, tag=pfx + "sums")

        for qo, qb in Q_BLOCKS:
            for j in range(4):
                g = j // 2
                hh0 = 2 * (j % 2)
                o_ps = psB.tile([128, QB], F32, tag="ops")
                for mc in range(8):
                    s_ps = psA.tile([128, 2, 512], F32, tag="big")
                    for i, hh in enumerate((hh0, hh0 + 1)):
                        p0 = 32 * hh
                        if USE_FP8:
                            mm(
                                s_ps[:, i, :qb],
                                kn8[g][p0 : p0 + 16, :, mc * 128 : (mc + 1) * 128],
                                qn8[g][p0 : p0 + 16, :, qo : qo + qb],
                                perf_mode=DR,
                                tile_position=(p0, 0),
                            )
                        else:
                            mm(
                                s_ps[:, i, :qb],
                                kn8[g][p0 : p0 + 32, 0, mc * 128 : (mc + 1) * 128],
                                qn8[g][p0 : p0 + 32, 0, qo : qo + qb],
                                tile_position=(p0, 0),
                            )
                    e_t = epool.tile([128, 2, QB], BF16, tag="E")
                    act(e_t[:, :, :qb], s_ps[:, :, :qb], AF.Exp)
                    a_t = apool.tile([128, 2, QB], BF16, tag="A")
                    eng = nc.gpsimd if (mc >= 6 or (mc >= 4 and j < 2)) else nc.vector
                    if USE_BRD:
                        eng.tensor_mul(
                            a_t[:, :, :qb],
                            e_t[:, :, :qb],
                            _brd2(expB_sb[mc][:, qo : qo + qb]),
                        )
                    else:
                        for i in range(2):
                            eng.tensor_mul(
                                a_t[:, i, :qb],
                                e_t[:, i, :qb],
                                expB_sb[mc][:, qo : qo + qb],
                            )
                    for i, hh in enumerate((hh0, hh0 + 1)):
                        h = 4 * g + hh
                        base = 64 * (hh % 2)
                        mm(
                            o_ps[base : base + 33, :qb],
                            vsb[mc][:, h, :],
                            a_t[:, i, :qb],
                            start=(mc == 0),
                            stop=(mc == 7),
                            tile_position=(0, base),
                        )
                nc.vector.tensor_copy(osb[j][:, qo : qo + qb], o_ps[:, :qb])
                # softmax denominators (ones-column rows 0 and 64)
                dmag(
                    sums[2 * j : 2 * j + 1, qo : qo + qb],
                    osb[j][0:1, qo : qo + qb],
                )
                dmag(
                    sums[2 * j + 1 : 2 * j + 2, qo : qo + qb],
                    osb[j][64:65, qo : qo + qb],
                )

        invS = wpool.tile([8, QH], BF16, tag=pfx + "invS")
        with nc.allow_low_precision(reason="bf16 inv-denominator feeds bf16 matmul"):
            nc.vector.reciprocal(invS[:], sums[:])

        # ---- normalize + fused output projection + residual ----
        for qo, qb in Q_BLOCKS:
            for j in range(4):
                psx = psB.tile([128, QB], F32, tag="ops")
                mm(psx[:, :qb], expP[:, j, :], invS[:, qo : qo + qb])
                nc.vector.tensor_mul(
                    osb[j][:, qo : qo + qb], osb[j][:, qo : qo + qb], psx[:, :qb]
                )
            for dt_i in range(2):
                ps = psB.tile([128, QB], F32, tag="ops")
                for j in range(4):
                    mm(
                        ps[:, :qb],
                        w2aug[:, j, dt_i * 128 : (dt_i + 1) * 128],
                        osb[j][:, qo : qo + qb],
                        start=(j == 0),
                        stop=(j == 3),
                    )
                r1 = wpool.tile([128, QB], F32, tag=pfx + f"res{dt_i}")
                nc.vector.scalar_tensor_tensor(
                    r1[:, :qb],
                    ps[:, :qb],
                    woutb[:, dt_i, :],
                    fmapT[:, dt_i, qo : qo + qb],
                    op0=mybir.AluOpType.add,
                    op1=mybir.AluOpType.add,
                )
                dma(out_dram[dt_i * 128 : (dt_i + 1) * 128, qo : qo + qb], r1[:, :qb])


def kernel(
    track_tokens,
    feature_map,
    feature_positions,
    spatial_bias,
    Wq,
    Wk,
    Wv,
    Wo,
    W_out_w,
    W_out_b,
    W_coord_w,
    W_coord_b,
):
    global _NC_CACHE
    inputs = dict(
        track_tokens=track_tokens,
        feature_map=feature_map,
        feature_positions=feature_positions,
        spatial_bias=spatial_bias,
        Wq=Wq,
        Wk=Wk,
        Wv=Wv,
        Wo=Wo,
        W_out_w=W_out_w,
        W_out_b=W_out_b,
        W_coord_w=W_coord_w,
        W_coord_b=W_coord_b,
    )
    in_maps = build_in_maps(inputs)
    if _NC_CACHE is None:
        _NC_CACHE = _build_nc()
    res = run_bass_kernel_spmd(_NC_CACHE, in_maps, core_ids=list(range(N_CORES)))
    return assemble_output([res.results[c]["out"] for c in range(N_CORES)])


def build_in_maps(inputs):
    consts = _host_constants(
        np.asarray(inputs["Wq"], np.float32),
        np.asarray(inputs["Wk"], np.float32),
        np.asarray(inputs["Wv"], np.float32),
        np.asarray(inputs["Wo"], np.float32),
        np.asarray(inputs["W_out_w"], np.float32),
        np.asarray(inputs["W_out_b"], np.float32),
        np.asarray(inputs["W_coord_w"], np.float32),
        np.asarray(inputs["W_coord_b"], np.float32),
    )
    track_tokens = np.asarray(inputs["track_tokens"], np.float32)
    feature_map = np.asarray(inputs["feature_map"], np.float32)
    feature_positions = np.asarray(inputs["feature_positions"], np.float32)
    spatial_bias = np.asarray(inputs["spatial_bias"], np.float32)
    expB = np.exp(spatial_bias)

    in_maps = []
    for c in range(N_CORES):
        t, half = c // 2, c % 2
        qsl = slice(half * QH, (half + 1) * QH)
        pos = feature_positions[t, qsl].T  # (2, QH)
        hi = np.rint(pos)
        lo = pos - hi
        posHL = np.stack(
            [hi[0], lo[0], hi[1], lo[1], np.ones_like(hi[0])]
        )  # (5, QH)
        m = dict(consts)
        m["tokT"] = _bf(track_tokens[t].T)
        one = np.ones_like(pos[0])
        m["posA"] = _bf(
            np.stack(
                [
                    np.stack([hi[0], lo[0], hi[0], lo[0], one]),
                    np.stack([hi[1], lo[1], hi[1], lo[1], one]),
                ]
            )
        )
        m["posHL"] = _bf(posHL)
        m["expB"] = _bf(expB[t][:, qsl])
        m["fmapT"] = _f32(feature_map[t, qsl].T)
        in_maps.append(m)
    return in_maps


def assemble_output(per_core):
    """per_core: sequence of 8 per-core 'out' arrays, each (DF, QH)."""
    out = np.empty((T, HW, DF), np.float32)
    for c in range(N_CORES):
        t, half = c // 2, c % 2
        qsl = slice(half * QH, (half + 1) * QH)
        out[t, qsl, :] = np.asarray(per_core[c]).T
    return out
